# revision 1
# baseline (speedup 1.0000x reference)
"""BiLSTM-CRF Viterbi decode on 8 Trainium2 NeuronCores.

Data-parallel over batch: each core handles 16 of 128 sequences.

Per-core phases:
  P0 embedding gather (indirect DMA, 128 rows per DMA)
  P1 PE-transpose x_rows [tok,E] -> x_T [E,tok]
  P2 bulk input projection xproj = Wih_g @ x_T (+bias) staged to DRAM
  P3 512 fused fwd+bwd LSTM rounds (gate-dim on partitions, [128,16] tiles)
  P4 emissions em = W_out @ h (PSUM chunks) staged to DRAM, read back b-major
  P5 Viterbi DP (DVE chain, 511 steps)
  P6 bulk argmax of backpointers (constant-stationary matmul + DVE)
  P7 backtrace (DVE chain, one-hot dot per step)

All matmuls fp32 (bf16 flips ~50 tags vs the fp32 reference).
"""

import numpy as np

import concourse.bacc as bacc
import concourse.bass as bass
import concourse.mybir as mybir
import concourse.tile as tile
from concourse.bass import IndirectOffsetOnAxis
from concourse.bass_utils import run_bass_kernel_spmd
from concourse.masks import make_identity

F32 = mybir.dt.float32
I32 = mybir.dt.int32
I8 = mybir.dt.int8
Alu = mybir.AluOpType
Act = mybir.ActivationFunctionType
AxX = mybir.AxisListType.X

B, T, V, E, H, K = 128, 512, 100000, 128, 128, 9
NC = 8
Bc = B // NC          # 16 sequences per core
TOK = Bc * T          # 8192 tokens per core, flat index bt = b*T + t (b-major)
NBLK = TOK // 128     # 64 gather/transpose blocks
G4 = 4
# gate order in weights: i, f, g, o (torch). psum cols per dir: i(0) f(16) o(32) g(48)
GOFF = {0: 0, 1: 16, 3: 32, 2: 48}
CH = 32               # LSTM rounds per xproj chunk
NCH = T // CH
AM_CH = 56            # bulk-argmax steps per chunk
AM_N = [AM_CH] * 9 + [511 - 9 * AM_CH]


def build_program():
    nc = bacc.Bacc(None, target_bir_lowering=False)

    # ---------------- dram parameters ----------------
    embed = nc.declare_dram_parameter("embed", [V, E], F32, isOutput=False)
    idx = nc.declare_dram_parameter("idx", [128, NBLK], I32, isOutput=False)
    whh_pack = nc.declare_dram_parameter("whh_pack", [128, 1024], F32, isOutput=False)
    wih_pack = nc.declare_dram_parameter("wih_pack", [128, 1024], F32, isOutput=False)
    bias_pack = nc.declare_dram_parameter("bias_pack", [128, 8], F32, isOutput=False)
    lens_b = nc.declare_dram_parameter("lens_b", [128, Bc], F32, isOutput=False)
    woutT = nc.declare_dram_parameter("woutT", [128, 18], F32, isOutput=False)
    bout_b = nc.declare_dram_parameter("bout_b", [128, K], F32, isOutput=False)
    start_b = nc.declare_dram_parameter("start_b", [Bc, K], F32, isOutput=False)
    end_b = nc.declare_dram_parameter("end_b", [Bc, K], F32, isOutput=False)
    trans_all = nc.declare_dram_parameter("trans_all", [Bc, 81], F32, isOutput=False)
    irev9_p = nc.declare_dram_parameter("irev9", [Bc, K], F32, isOutput=False)
    iota9_p = nc.declare_dram_parameter("iota9", [Bc, K], F32, isOutput=False)
    mask_dp = nc.declare_dram_parameter("mask_dp", [Bc, T], F32, isOutput=False)
    mask_i8p = nc.declare_dram_parameter("mask_i8", [Bc, T], I8, isOutput=False)
    selAB = nc.declare_dram_parameter("selAB", [48, 144], F32, isOutput=False)
    trans_tiled = nc.declare_dram_parameter("trans_tiled", [K, AM_CH * K], F32, isOutput=False)
    iota_rev_am = nc.declare_dram_parameter("iota_rev_am", [72, AM_CH * K], F32, isOutput=False)
    iota_j72 = nc.declare_dram_parameter("iota_j72", [72, 1], F32, isOutput=False)
    mask_bj = nc.declare_dram_parameter("mask_bj", [72, 2 * T], I8, isOutput=False)
    tags_out = nc.declare_dram_parameter("tags", [Bc, T], I32, isOutput=True)
    dbg_xt = nc.declare_dram_parameter("dbg_xt", [128, 256], F32, isOutput=True)
    dbg_xp = nc.declare_dram_parameter("dbg_xp", [128, 512], F32, isOutput=True)
    dbg_em = nc.declare_dram_parameter("dbg_em", [Bc, T * K], F32, isOutput=True)
    dbg_hist = nc.declare_dram_parameter("dbg_hist", [Bc, 511 * K], F32, isOutput=True)
    dbg_idx = nc.declare_dram_parameter("dbg_idx", [Bc, 511 * K], F32, isOutput=True)
    dbg_hf = nc.declare_dram_parameter("dbg_hf", [128, 512], F32, isOutput=True)
    dbg_hb = nc.declare_dram_parameter("dbg_hb", [128, 512], F32, isOutput=True)

    # ---------------- dram internals ----------------
    xproj_dram = nc.dram_tensor("xproj_dram", [2, G4, Bc, 128, T], F32)
    em_dram = nc.dram_tensor("em_dram", [TOK, K], F32)

    with tile.TileContext(nc) as tc:
        with (
            tc.tile_pool(name="big", bufs=1) as big,
            tc.tile_pool(name="xp", bufs=2) as xpp,
            tc.tile_pool(name="consts", bufs=1) as cst,
            tc.tile_pool(name="small", bufs=2) as sm,
        ):
            # ---------- constants ----------
            idx_sb = cst.tile([128, NBLK], I32)
            nc.sync.dma_start(out=idx_sb[:], in_=idx[:])
            whh_sb = cst.tile([128, 1024], F32)
            nc.sync.dma_start(out=whh_sb[:], in_=whh_pack[:])
            wih_sb = cst.tile([128, 1024], F32)
            nc.sync.dma_start(out=wih_sb[:], in_=wih_pack[:])
            bias_sb = cst.tile([128, 8], F32)
            nc.sync.dma_start(out=bias_sb[:], in_=bias_pack[:])
            lens_sb = cst.tile([128, Bc], F32)
            nc.sync.dma_start(out=lens_sb[:], in_=lens_b[:])
            woutT_sb = cst.tile([128, 18], F32)
            nc.sync.dma_start(out=woutT_sb[:], in_=woutT[:])
            bout_sb = cst.tile([128, K], F32)
            nc.sync.dma_start(out=bout_sb[:], in_=bout_b[:])
            ident = cst.tile([128, 128], F32)
            make_identity(nc, ident[:])
            bias_row = None  # placeholder to keep diff local

            # PE "absorber" ops: self-loading (fp32) matmuls may carry at
            # most ONE sync wait in walrus codegen. These tiny ops advance
            # PE's vector clock over one-time deps (identity from Pool,
            # const-weight DMA lanes) so real matmuls each need <=1 wait.
            psp_cm = tc.tile_pool(name="psglob", bufs=1, space="PSUM")
            psp = psp_cm.__enter__()
            pq1 = psp.tile([128, 128], F32, tag="pq1", name="pq1")
            pq2 = psp.tile([128, 128], F32, tag="pq2", name="pq2")
            pw1 = psp.tile([128, 512], F32, tag="pw1", name="pw1")
            pw2 = psp.tile([128, 512], F32, tag="pw2", name="pw2")
            pw3 = psp.tile([128, 512], F32, tag="pw3", name="pw3")
            nc.tensor.transpose(out=pq1[:], in_=ident[:], identity=ident[:])
            for cst_ap in (wih_sb[:, 0:1], whh_sb[:, 0:1], woutT_sb[:, 0:1]):
                nc.tensor.matmul(out=pq2[0:1, 0:1], lhsT=cst_ap,
                                 rhs=ident[:, 0:1], start=True, stop=True)

            # ---------- P0: gather ----------
            x_rows = []
            with tc.tile_pool(name="xr", bufs=24) as xrp:
                for g in range(NBLK):
                    xr = xrp.tile([128, 128], F32, tag="xr")
                    nc.gpsimd.indirect_dma_start(
                        out=xr[:],
                        out_offset=None,
                        in_=embed[:],
                        in_offset=IndirectOffsetOnAxis(
                            ap=idx_sb[:, g:g + 1], axis=0),
                    )
                    x_rows.append(xr)

                # ---------- P1: transpose ----------
                # All PE-facing producers routed through DVE so each
                # self-loading matmul needs a single sync wait (walrus limit):
                # relay gathered blocks DVE, 1-elem PSUM memset flips the
                # slot's last-writer to DVE, output copies DVE.
                x_T = big.tile([128, TOK], F32, tag="bigB")
                with tc.tile_pool(name="xrel", bufs=4) as xrelp:
                    psts = [pq1, pq2]
                    for g in range(NBLK):
                        xrel = xrelp.tile([128, 128], F32, tag="xrel")
                        nc.vector.tensor_tensor(
                            out=xrel[:], in0=x_rows[g][:], in1=x_rows[g][:],
                            op=Alu.max)
                        pst = psts[g % 2]
                        nc.tensor.transpose(
                            out=pst[:], in_=xrel[:], identity=ident[:])
                        nc.vector.tensor_copy(
                            out=x_T[:, g * 128:(g + 1) * 128], in_=pst[:])

            # ---------- P2: bulk xproj ----------
            if True:
                ps2s = [pw1[:], pw2[:], pw3[:]]
                n2 = 0
                for d in range(2):
                    for g in range(G4):
                        lhsT = wih_sb[:, (d * G4 + g) * 128:(d * G4 + g + 1) * 128]
                        for b in range(Bc):
                            ps2 = ps2s[n2 % 3]
                            n2 += 1
                            nc.tensor.matmul(
                                out=ps2, lhsT=lhsT,
                                rhs=x_T[:, b * T:(b + 1) * T],
                                start=True, stop=True)
                            xp_sb = sm.tile([128, 512], F32, tag="xp_out")
                            nc.vector.tensor_scalar(
                                out=xp_sb[:], in0=ps2,
                                scalar1=bias_sb[:, d * G4 + g:d * G4 + g + 1],
                                scalar2=None, op0=Alu.add)
                            # store at PSUM block position (i,f,o,g order)
                            nc.sync.dma_start(
                                out=xproj_dram[d, GOFF[g] // 16, b],
                                in_=xp_sb[:])

            nc.sync.dma_start(out=dbg_xt[:], in_=x_T[:, 0:256])
            nc.sync.dma_start(out=dbg_xp[:], in_=xproj_dram[0, 0, 0])

            # ---------- P3: LSTM ----------
            h_f = big.tile([128, TOK], F32, tag="bigA")
            h_b = big.tile([128, TOK], F32, tag="bigB")
            h0 = cst.tile([128, Bc], F32)
            nc.vector.memset(h0[:], 0.0)
            c_st = cst.tile([128, 2 * Bc], F32)
            nc.vector.memset(c_st[:], 0.0)

            if True:
                ps3 = pq1
                xp_tiles = {}
                for r in range(T):
                    tf, tb = r, T - 1 - r
                    c = r // CH
                    if r % CH == 0:
                        for d, cc in ((0, c), (1, NCH - 1 - c)):
                            xt = xpp.tile([128, G4 * Bc * CH], F32, tag=f"xpc{d}")
                            src = xproj_dram[d][:, :, :, cc * CH:(cc + 1) * CH]
                            src = src.transpose([2, 0, 1, 3])
                            dst = xt[:].rearrange(
                                "p (g b t) -> p g b t", g=G4, b=Bc, t=CH)
                            nc.sync.dma_start(out=dst, in_=src)
                            xp_tiles[d] = xt

                    for d in range(2):
                        if r == 0:
                            hprev = h0[:]
                        elif d == 0:
                            hprev = h_f[:, tf - 1::T]
                        else:
                            hprev = h_b[:, tb + 1::T]
                        for g in range(G4):
                            lhsT = whh_sb[
                                :, (d * G4 + g) * 128:(d * G4 + g + 1) * 128]
                            nc.tensor.matmul(
                                out=ps3[:, d * 64 + GOFF[g]:d * 64 + GOFF[g] + Bc],
                                lhsT=lhsT, rhs=hprev, start=True, stop=True)
                    gsb = sm.tile([128, 128], F32, tag="gates")
                    for d, tt in ((0, tf), (1, tb)):
                        xsl = xp_tiles[d][:].rearrange(
                            "p (g b t) -> p g b t", g=G4, b=Bc, t=CH
                        )[:, :, :, tt % CH]
                        nc.vector.tensor_tensor(
                            out=gsb[:, d * 64:d * 64 + 64],
                            in0=ps3[:, d * 64:d * 64 + 64], in1=xsl, op=Alu.add)
                    sig = sm.tile([128, 96], F32, tag="sig")
                    nc.scalar.activation(
                        out=sig[:].rearrange(
                            "p (q d b) -> p d q b", q=3, d=2, b=Bc),
                        in_=gsb[:].rearrange("p (d x) -> p d x", d=2)[:, :, 0:48],
                        func=Act.Sigmoid)
                    tg = sm.tile([128, 2 * Bc], F32, tag="tg")
                    nc.scalar.activation(
                        out=tg[:],
                        in_=gsb[:].rearrange("p (d x) -> p d x", d=2)[:, :, 48:64],
                        func=Act.Tanh)
                    t1 = sm.tile([128, 2 * Bc], F32, tag="t1")
                    nc.gpsimd.tensor_tensor(
                        out=t1[:], in0=sig[:, 0:2 * Bc], in1=tg[:], op=Alu.mult)
                    t2 = sm.tile([128, 2 * Bc], F32, tag="t2")
                    nc.vector.tensor_tensor(
                        out=t2[:], in0=sig[:, 2 * Bc:4 * Bc], in1=c_st[:],
                        op=Alu.mult)
                    nc.vector.tensor_tensor(
                        out=c_st[:], in0=t1[:], in1=t2[:], op=Alu.add)
                    tcx = sm.tile([128, 2 * Bc], F32, tag="tc")
                    nc.scalar.activation(out=tcx[:], in_=c_st[:], func=Act.Tanh)
                    mt = sm.tile([128, Bc], F32, tag="mt")
                    nc.gpsimd.tensor_scalar(
                        out=mt[:], in0=lens_sb[:], scalar1=float(tb),
                        scalar2=None, op0=Alu.is_gt)
                    nc.vector.tensor_tensor(
                        out=h_f[:, tf::T], in0=sig[:, 4 * Bc:5 * Bc],
                        in1=tcx[:, 0:Bc], op=Alu.mult)
                    hbt = sm.tile([128, Bc], F32, tag="hbt")
                    nc.gpsimd.tensor_tensor(
                        out=hbt[:], in0=sig[:, 5 * Bc:6 * Bc],
                        in1=tcx[:, Bc:2 * Bc], op=Alu.mult)
                    nc.vector.tensor_tensor(
                        out=h_b[:, tb::T], in0=hbt[:], in1=mt[:], op=Alu.mult)
                    nc.gpsimd.tensor_tensor(
                        out=c_st[:, Bc:2 * Bc], in0=c_st[:, Bc:2 * Bc],
                        in1=mt[:], op=Alu.mult)

            nc.sync.dma_start(out=dbg_hf[:], in_=h_f[:, 0:512])
            nc.sync.dma_start(out=dbg_hb[:], in_=h_b[:, 0:512])

            # ---------- P4: emissions ----------
            if True:
                ps4s = [pq2[:, 0:K], pq1[:, 0:K]]
                for ch in range(NBLK):
                    ps4 = ps4s[ch % 2]
                    nc.tensor.matmul(
                        out=ps4, lhsT=h_f[:, ch * 128:(ch + 1) * 128],
                        rhs=woutT_sb[:, 0:K], start=True, stop=False)
                    nc.tensor.matmul(
                        out=ps4, lhsT=h_b[:, ch * 128:(ch + 1) * 128],
                        rhs=woutT_sb[:, K:2 * K], start=False, stop=True)
                    em_sb = sm.tile([128, K], F32, tag="em_sb")
                    nc.vector.tensor_tensor(
                        out=em_sb[:], in0=ps4, in1=bout_sb[:], op=Alu.add)
                    nc.sync.dma_start(
                        out=em_dram[ch * 128:(ch + 1) * 128, :], in_=em_sb[:])

            # ---------- P5: Viterbi DP ----------
            em_dp = big.tile([Bc, T * K], F32, tag="em_idx")
            nc.sync.dma_start(
                out=em_dp[:],
                in_=em_dram[:].rearrange("(b x) k -> b (x k)", b=Bc))
            trans_sb = cst.tile([Bc, 81], F32)
            nc.sync.dma_start(out=trans_sb[:], in_=trans_all[:])
            irev9_sb = cst.tile([Bc, K], F32)
            nc.sync.dma_start(out=irev9_sb[:], in_=irev9_p[:])
            iota9_sb = cst.tile([Bc, K], F32)
            nc.sync.dma_start(out=iota9_sb[:], in_=iota9_p[:])
            start_sb = cst.tile([Bc, K], F32)
            nc.sync.dma_start(out=start_sb[:], in_=start_b[:])
            end_sb = cst.tile([Bc, K], F32)
            nc.sync.dma_start(out=end_sb[:], in_=end_b[:])
            mask_sb = cst.tile([Bc, T], F32)
            nc.sync.dma_start(out=mask_sb[:], in_=mask_dp[:])
            mask_i8_sb = cst.tile([Bc, T], I8)
            nc.sync.dma_start(out=mask_i8_sb[:], in_=mask_i8p[:])

            S = cst.tile([Bc, K], F32)
            hist = big.tile([Bc, 511 * K], F32, tag="hist")
            cand = sm.tile([Bc, 81], F32, tag="cand")
            mx = sm.tile([Bc, K], F32, tag="mx")
            snew = sm.tile([Bc, K], F32, tag="snew")

            nc.vector.tensor_tensor(
                out=S[:], in0=em_dp[:, 0:K], in1=start_sb[:], op=Alu.add)
            for t in range(1, T):
                nc.scalar.copy(out=hist[:, (t - 1) * K:t * K], in_=S[:])
                nc.vector.tensor_tensor(
                    out=cand[:].rearrange("p (j i) -> p j i", j=K),
                    in0=S[:].unsqueeze(1).to_broadcast([Bc, K, K]),
                    in1=trans_sb[:].rearrange("p (j i) -> p j i", j=K),
                    op=Alu.add)
                nc.vector.tensor_reduce(
                    out=mx[:], in_=cand[:].rearrange("p (j i) -> p j i", j=K),
                    axis=AxX, op=Alu.max)
                nc.vector.tensor_tensor(
                    out=snew[:], in0=mx[:], in1=em_dp[:, t * K:(t + 1) * K],
                    op=Alu.add)
                nc.vector.copy_predicated(
                    out=S[:], mask=mask_i8_sb[:, t:t + 1].to_broadcast([Bc, K]),
                    data=snew[:])

            tags_f = big.tile([Bc, T], F32, tag="tags_f")
            nc.vector.tensor_tensor(
                out=S[:], in0=S[:], in1=end_sb[:], op=Alu.add)
            m1 = sm.tile([Bc, 1], F32, tag="m1")
            nc.vector.tensor_reduce(out=m1[:], in_=S[:], axis=AxX, op=Alu.max)
            eqv = sm.tile([Bc, K], F32, tag="eqv")
            nc.vector.tensor_tensor(
                out=eqv[:], in0=S[:], in1=m1[:].to_broadcast([Bc, K]),
                op=Alu.is_equal)
            nc.vector.tensor_tensor(
                out=eqv[:], in0=eqv[:], in1=irev9_sb[:], op=Alu.mult)
            r1 = sm.tile([Bc, 1], F32, tag="r1")
            nc.vector.tensor_reduce(out=r1[:], in_=eqv[:], axis=AxX, op=Alu.max)
            nc.vector.tensor_scalar(
                out=tags_f[:, T - 1:T], in0=r1[:], scalar1=-1.0, scalar2=8.0,
                op0=Alu.mult, op1=Alu.add)

            # ---------- P6: bulk argmax ----------
            selAB_dma = cst.tile([48, 144], F32)
            nc.sync.dma_start(out=selAB_dma[:], in_=selAB[:])
            selAB_sb = cst.tile([48, 144], F32)
            nc.vector.tensor_copy(out=selAB_sb[:], in_=selAB_dma[:])
            ttl_dma = cst.tile([K, AM_CH * K], F32)
            nc.sync.dma_start(out=ttl_dma[:], in_=trans_tiled[:])
            iram_sb = cst.tile([72, AM_CH * K], F32)
            nc.sync.dma_start(out=iram_sb[:], in_=iota_rev_am[:])
            ij72_sb = cst.tile([72, 1], F32)
            nc.sync.dma_start(out=ij72_sb[:], in_=iota_j72[:])
            mask_bj_sb = cst.tile([72, 2 * T], I8)
            nc.sync.dma_start(out=mask_bj_sb[:], in_=mask_bj[:])
            Rrhs = cst.tile([48, AM_CH * K], F32)
            nc.vector.tensor_copy(out=Rrhs[0:K, :], in_=ttl_dma[:])

            # idx_dp: [b, (j, s)] layout, s = 0..510 for steps t = 1..511
            idx_dp = big.tile([Bc, K * 511], F32, tag="em_idx")

            if True:
                psA = pw1[0:72, 0:AM_CH * K]
                psB = pw2[0:72, 0:AM_CH * K]
                s0 = 0
                for ci, ns in enumerate(AM_N):
                    W = ns * K
                    nc.vector.tensor_tensor(
                        out=Rrhs[32:48, 0:W],
                        in0=hist[:, s0 * K:(s0 + ns) * K],
                        in1=hist[:, s0 * K:(s0 + ns) * K], op=Alu.max)
                    nc.tensor.matmul(out=psA[:, 0:W], lhsT=selAB_sb[:, 0:72],
                                     rhs=Rrhs[:, 0:W], start=True, stop=True)
                    nc.tensor.matmul(out=psB[:, 0:W], lhsT=selAB_sb[:, 72:144],
                                     rhs=Rrhs[:, 0:W], start=True, stop=True)

                    for hi, psx in ((0, psA), (1, psB)):
                        half = "AB"[hi]
                        view = psx[:, 0:W].rearrange("p (t i) -> p t i", i=K)
                        mxa = sm.tile([72, AM_CH], F32, tag=f"mx{half}")
                        nc.vector.tensor_reduce(
                            out=mxa[:, 0:ns], in_=view, axis=AxX, op=Alu.max)
                        eqa = sm.tile([72, AM_CH * K], F32, tag=f"eq{half}")
                        nc.vector.tensor_tensor(
                            out=eqa[:, 0:W].rearrange("p (t i) -> p t i", i=K),
                            in0=view,
                            in1=mxa[:, 0:ns].unsqueeze(2).to_broadcast(
                                [72, ns, K]),
                            op=Alu.is_equal)
                        nc.vector.tensor_tensor(
                            out=eqa[:, 0:W], in0=eqa[:, 0:W],
                            in1=iram_sb[:, 0:W], op=Alu.mult)
                        ra = sm.tile([72, AM_CH], F32, tag=f"r{half}")
                        nc.vector.tensor_reduce(
                            out=ra[:, 0:ns],
                            in_=eqa[:, 0:W].rearrange("p (t i) -> p t i", i=K),
                            axis=AxX, op=Alu.max)
                        # ia = 8 - ra where valid, = j at padded steps
                        ia = sm.tile([72, AM_CH], F32, tag=f"i{half}")
                        nc.vector.tensor_tensor(
                            out=ia[:, 0:ns],
                            in0=ij72_sb[:].to_broadcast([72, ns]),
                            in1=ij72_sb[:].to_broadcast([72, ns]), op=Alu.max)
                        ra2 = sm.tile([72, AM_CH], F32, tag=f"r2{half}")
                        nc.vector.tensor_scalar(
                            out=ra2[:, 0:ns], in0=ra[:, 0:ns], scalar1=-1.0,
                            scalar2=8.0, op0=Alu.mult, op1=Alu.add)
                        nc.vector.copy_predicated(
                            out=ia[:, 0:ns],
                            mask=mask_bj_sb[:, hi * T + s0 + 1:
                                            hi * T + s0 + 1 + ns],
                            data=ra2[:, 0:ns])
                        # regroup [(b,j), t] -> [b, (j, s0+t)] via sbuf dma
                        nc.sync.dma_start(
                            out=idx_dp[hi * 8:(hi + 1) * 8, :].rearrange(
                                "p (j s) -> p j s", j=K)[:, :, s0:s0 + ns],
                            in_=ia[:, 0:ns])
                    s0 += ns

            # ---------- P7: backtrace ----------
            oh = sm.tile([Bc, K], F32, tag="oh")
            for s in range(T - 2, -1, -1):
                nc.vector.tensor_scalar(
                    out=oh[:], in0=iota9_sb[:], scalar1=tags_f[:, s + 1:s + 2],
                    scalar2=None, op0=Alu.is_equal)
                nc.vector.tensor_tensor(
                    out=oh[:], in0=oh[:],
                    in1=idx_dp[:].rearrange("p (j s) -> p j s", j=K)[:, :, s],
                    op=Alu.mult)
                nc.vector.tensor_reduce(
                    out=tags_f[:, s:s + 1], in_=oh[:], axis=AxX, op=Alu.add)
            nc.vector.tensor_tensor(
                out=tags_f[:], in0=tags_f[:], in1=mask_sb[:], op=Alu.mult)
            tags_i = big.tile([Bc, T], I32, tag="tags_i")
            nc.vector.tensor_copy(out=tags_i[:], in_=tags_f[:])
            nc.sync.dma_start(out=tags_out[:], in_=tags_i[:])
            nc.sync.dma_start(out=dbg_em[:], in_=em_dp[:])
            nc.sync.dma_start(out=dbg_hist[:], in_=hist[:])
            nc.sync.dma_start(out=dbg_idx[:], in_=idx_dp[:])
            psp_cm.__exit__(None, None, None)

    nc.finalize()
    return nc


_NC_CACHE = None


def _get_program():
    global _NC_CACHE
    if _NC_CACHE is None:
        _NC_CACHE = build_program()
    return _NC_CACHE


def make_in_maps(sentences, lengths, embed, Wih_f, Whh_f, bih_f, bhh_f,
                 Wih_b, Whh_b, bih_b, bhh_b, W_out, b_out, start_t, end_t,
                 trans):
    sentences = np.ascontiguousarray(sentences, dtype=np.int32)
    embed = np.ascontiguousarray(embed, dtype=np.float32)
    lengths = np.asarray(lengths)

    whh_pack = np.zeros((128, 1024), np.float32)
    wih_pack = np.zeros((128, 1024), np.float32)
    bias_pack = np.zeros((128, 8), np.float32)
    for d, (Wih, Whh, bi, bh) in enumerate(
            ((Wih_f, Whh_f, bih_f, bhh_f), (Wih_b, Whh_b, bih_b, bhh_b))):
        for g in range(G4):
            whh_pack[:, (d * G4 + g) * 128:(d * G4 + g + 1) * 128] = \
                np.asarray(Whh)[g * 128:(g + 1) * 128, :].T
            wih_pack[:, (d * G4 + g) * 128:(d * G4 + g + 1) * 128] = \
                np.asarray(Wih)[g * 128:(g + 1) * 128, :].T
            bias_pack[:, d * G4 + g] = \
                (np.asarray(bi) + np.asarray(bh))[g * 128:(g + 1) * 128]

    W_out = np.asarray(W_out, np.float32)
    woutT = np.zeros((128, 18), np.float32)
    woutT[:, 0:K] = W_out[:, :128].T
    woutT[:, K:2 * K] = W_out[:, 128:].T
    bout_bc = np.broadcast_to(
        np.asarray(b_out, np.float32)[None, :], (128, K)).copy()
    start_bc = np.broadcast_to(
        np.asarray(start_t, np.float32)[None, :], (Bc, K)).copy()
    end_bc = np.broadcast_to(
        np.asarray(end_t, np.float32)[None, :], (Bc, K)).copy()

    trans_np = np.asarray(trans, np.float32)
    trans_flat = trans_np.T.reshape(-1)  # [(j,i)] = trans[i,j]
    trans_allv = np.broadcast_to(trans_flat[None, :], (Bc, 81)).copy()
    ii = np.arange(K, dtype=np.float32)
    irev9 = np.broadcast_to((8.0 - ii)[None, :], (Bc, K)).copy()
    iota9_np = np.broadcast_to(ii[None, :], (Bc, K)).copy()

    selAB_np = np.zeros((48, 144), np.float32)
    for half in range(2):
        for m in range(72):
            b_loc, j = divmod(m, K)
            selAB_np[32 + half * 8 + b_loc, half * 72 + m] = 1.0
            selAB_np[j, half * 72 + m] = 1.0
    ttl = np.zeros((K, AM_CH * K), np.float32)
    for jp in range(K):
        ttl[jp] = np.tile(trans_np[:, jp], AM_CH)
    iram = np.broadcast_to(
        np.tile(8.0 - ii, AM_CH)[None, :], (72, AM_CH * K)).copy()
    ij72 = (np.arange(72, dtype=np.float32) % K)[:, None].copy()
    tt = np.arange(T)

    in_maps = []
    for c in range(NC):
        sl = slice(c * Bc, (c + 1) * Bc)
        sents_c = sentences[sl]
        lens_c = np.asarray(lengths[sl], np.float32)
        idx_np = np.zeros((128, NBLK), np.int32)
        p = np.arange(128)
        for g in range(NBLK):
            bt = g * 128 + p
            idx_np[:, g] = sents_c[bt // T, bt % T]
        lens_bc = np.broadcast_to(lens_c[None, :], (128, Bc)).copy()
        mask_np = (tt[None, :] < lens_c[:, None]).astype(np.float32)
        # mask_bj[(half, b_loc, j), t] = mask[half*8 + b_loc, t]
        mbj = np.repeat(mask_np, K, axis=0).astype(np.int8)  # [144, T]
        mask_bj_np = np.concatenate([mbj[:72], mbj[72:]], axis=1)  # [72, 2T]
        in_maps.append({
            "embed": embed,
            "idx": idx_np,
            "whh_pack": whh_pack, "wih_pack": wih_pack, "bias_pack": bias_pack,
            "lens_b": lens_bc,
            "woutT": woutT, "bout_b": bout_bc,
            "start_b": start_bc, "end_b": end_bc,
            "trans_all": trans_allv, "irev9": irev9, "iota9": iota9_np,
            "mask_dp": mask_np, "mask_i8": mask_np.astype(__import__("numpy").int8),
            "selAB": selAB_np, "trans_tiled": ttl, "iota_rev_am": iram,
            "iota_j72": ij72, "mask_bj": mask_bj_np,
        })
    return in_maps


def run(inputs, trace=False, **kw):
    nc = _get_program()
    in_maps = make_in_maps(**inputs)
    res = run_bass_kernel_spmd(nc, in_maps, list(range(NC)), trace=trace, **kw)
    tags = np.concatenate([r["tags"] for r in res.results], axis=0)
    return tags.astype(np.int32), res


def kernel(**inputs):
    tags, _ = run(inputs)
    return tags



# revision 31
# speedup vs baseline: 1.0266x; 1.0266x over previous
"""BiLSTM-CRF Viterbi decode on 8 Trainium2 NeuronCores.

Data-parallel over batch: each core handles 16 of 128 sequences.

Per-core phases:
  P0 embedding gather (indirect DMA, 128 rows per DMA)
  P1 PE-transpose x_rows [tok,E] -> x_T [E,tok] and x_Trev (time-reversed
     per sequence, via anti-diagonal identity)
  P2 bulk input projection xproj = Wih_g @ x_T (+bias) staged to DRAM,
     fp32r matmuls (N=512); bwd direction projected from x_Trev so its
     DRAM layout is s-ordered (s = T-1-t)
  P3 512 fused fwd+bwd LSTM rounds (gate-dim on partitions, [128,16]
     tiles); one [128,128] xproj+psum add per round; ping-pong PSUM
  P4 emissions em^T = W_out @ h as [9,512] fp32r matmuls staged to DRAM
  P5 forward (M-form) and backward (beta) Viterbi DPs on DVE, reading
     precombined slabs B[s][i,j] = trans +/- em built on GpSimd; masked
     steps become max-plus identity so no per-step predication
  P6 tags = argmax_i(M_t + em_t + beta_t), bulk DVE ops

All matmuls fp32/fp32r (bf16 flips ~50 tags vs the fp32 reference).
"""

import numpy as np

import concourse.bacc as bacc
import concourse.bass as bass
import concourse.mybir as mybir
import concourse.tile as tile
from concourse.bass import IndirectOffsetOnAxis
from concourse.bass_utils import run_bass_kernel_spmd
from concourse.masks import make_identity

F32 = mybir.dt.float32
F32R = mybir.dt.float32r
I32 = mybir.dt.int32
Alu = mybir.AluOpType
Act = mybir.ActivationFunctionType
AxX = mybir.AxisListType.X

B, T, V, E, H, K = 128, 512, 100000, 128, 128, 9
NC = 8
Bc = B // NC          # 16 sequences per core
TOK = Bc * T          # 8192 tokens per core, flat index bt = b*T + t (b-major)
NBLK = TOK // 128     # 64 gather/transpose blocks
G4 = 4
# gate order in weights: i, f, g, o (torch). psum cols per dir: i(0) f(16) o(32) g(48)
GOFF = {0: 0, 1: 16, 3: 32, 2: 48}
CH = 32               # LSTM rounds per xproj chunk
NCH = T // CH
DPCH = 32             # Viterbi DP steps per slab chunk
NEG = -1.0e9


def f32(ap):
    return ap.bitcast(F32)


def build_program():
    nc = bacc.Bacc(None, target_bir_lowering=False)

    # ---------------- dram parameters ----------------
    embed = nc.declare_dram_parameter("embed", [V, E], F32, isOutput=False)
    idx = nc.declare_dram_parameter("idx", [128, NBLK], I32, isOutput=False)
    whh_pack = nc.declare_dram_parameter("whh_pack", [128, 1024], F32, isOutput=False)
    wih_pack = nc.declare_dram_parameter("wih_pack", [128, 1024], F32, isOutput=False)
    bias_pack = nc.declare_dram_parameter("bias_pack", [128, 8], F32, isOutput=False)
    lens_b = nc.declare_dram_parameter("lens_b", [128, Bc], F32, isOutput=False)
    woutT = nc.declare_dram_parameter("woutT", [128, 18], F32, isOutput=False)
    bout9 = nc.declare_dram_parameter("bout9", [K, 1], F32, isOutput=False)
    rev_id = nc.declare_dram_parameter("rev_id", [128, 128], F32, isOutput=False)
    transT9 = nc.declare_dram_parameter("transT9", [Bc, 81], F32, isOutput=False)
    trans9 = nc.declare_dram_parameter("trans9", [Bc, 81], F32, isOutput=False)
    identM9 = nc.declare_dram_parameter("identM9", [Bc, 81], F32, isOutput=False)
    start9 = nc.declare_dram_parameter("start9", [Bc, K], F32, isOutput=False)
    end9 = nc.declare_dram_parameter("end9", [Bc, K], F32, isOutput=False)
    irev9_p = nc.declare_dram_parameter("irev9", [Bc, K], F32, isOutput=False)
    mask_dp = nc.declare_dram_parameter("mask_dp", [Bc, T], F32, isOutput=False)
    invm_dp = nc.declare_dram_parameter("invm_dp", [Bc, T], F32, isOutput=False)
    tags_out = nc.declare_dram_parameter("tags", [Bc, T], I32, isOutput=True)

    # ---------------- dram internals ----------------
    xproj_dram = nc.dram_tensor("xproj_dram", [2, G4, Bc, 128, T], F32)
    emT_dram = nc.dram_tensor("emT_dram", [K, TOK], F32)

    with tile.TileContext(nc) as tc:
        with (
            tc.tile_pool(name="big", bufs=1) as big,
            tc.tile_pool(name="consts", bufs=1) as cst,
            tc.tile_pool(name="small", bufs=3) as sm,
        ):
            # ---------- constants ----------
            idx_sb = cst.tile([128, NBLK], I32)
            nc.sync.dma_start(out=idx_sb[:], in_=idx[:])
            whh_sb = cst.tile([128, 1024], F32)
            nc.sync.dma_start(out=whh_sb[:], in_=whh_pack[:])
            wih_sb = cst.tile([128, 1024], F32)
            nc.sync.dma_start(out=wih_sb[:], in_=wih_pack[:])
            bias_sb = cst.tile([128, 8], F32)
            nc.sync.dma_start(out=bias_sb[:], in_=bias_pack[:])
            lens_sb = cst.tile([128, Bc], F32)
            nc.sync.dma_start(out=lens_sb[:], in_=lens_b[:])
            woutT_sb = cst.tile([128, 18], F32)
            nc.sync.dma_start(out=woutT_sb[:], in_=woutT[:])
            # device-rounded fp32r copies (the fp32r matmul path requires
            # its operands to be produced pre-rounded to fp32r)
            wihr_sb = cst.tile([128, 1024], F32R)
            nc.vector.tensor_copy(out=wihr_sb[:], in_=wih_sb[:])
            woutTr_sb = cst.tile([128, 18], F32R)
            nc.vector.tensor_copy(out=woutTr_sb[:], in_=woutT_sb[:])
            bout9_sb = cst.tile([K, 1], F32)
            nc.sync.dma_start(out=bout9_sb[:], in_=bout9[:])
            rev_dma = cst.tile([128, 128], F32)
            nc.sync.dma_start(out=rev_dma[:], in_=rev_id[:])
            rev_sb = cst.tile([128, 128], F32)
            nc.vector.tensor_copy(out=rev_sb[:], in_=rev_dma[:])
            ident = cst.tile([128, 128], F32)
            make_identity(nc, ident[:])

            # DP constants (p0-15)
            transT9_sb = cst.tile([Bc, 81], F32)
            nc.sync.dma_start(out=transT9_sb[:], in_=transT9[:])
            trans9_sb = cst.tile([Bc, 81], F32)
            nc.sync.dma_start(out=trans9_sb[:], in_=trans9[:])
            identM9_sb = cst.tile([Bc, 81], F32)
            nc.sync.dma_start(out=identM9_sb[:], in_=identM9[:])
            start9_sb = cst.tile([Bc, K], F32)
            nc.sync.dma_start(out=start9_sb[:], in_=start9[:])
            end9_sb = cst.tile([Bc, K], F32)
            nc.sync.dma_start(out=end9_sb[:], in_=end9[:])
            irev9_sb = cst.tile([Bc, K], F32)
            nc.sync.dma_start(out=irev9_sb[:], in_=irev9_p[:])
            mask_sb = cst.tile([Bc, T], F32)
            nc.sync.dma_start(out=mask_sb[:], in_=mask_dp[:])
            invm_sb = cst.tile([Bc, T], F32)
            nc.sync.dma_start(out=invm_sb[:], in_=invm_dp[:])

            # PE "absorber" ops: self-loading (fp32/fp32r) matmuls may carry
            # at most ONE sync wait in walrus codegen. These tiny ops advance
            # PE's vector clock over one-time deps (identity from Pool,
            # const-weight DMA lanes) so real matmuls each need <=1 wait.
            psp_cm = tc.tile_pool(name="psglob", bufs=1, space="PSUM")
            psp = psp_cm.__enter__()
            pq1 = psp.tile([128, 512], F32, tag="pq1", name="pq1")
            pq2 = psp.tile([128, 512], F32, tag="pq2", name="pq2")
            pw1 = psp.tile([128, 512], F32, tag="pw1", name="pw1")
            pw2 = psp.tile([128, 512], F32, tag="pw2", name="pw2")
            pw3 = psp.tile([128, 512], F32, tag="pw3", name="pw3")
            nc.tensor.transpose(out=pq1[:, 0:128], in_=ident[:], identity=ident[:])
            nc.tensor.transpose(out=pq2[:, 0:128], in_=rev_sb[:], identity=ident[:])
            nc.tensor.matmul(out=pq2[0:1, 0:1], lhsT=whh_sb[:, 0:1],
                             rhs=ident[:, 0:1], start=True, stop=True)

            # ---------- P0: gather ----------
            x_T = big.tile([128, TOK], F32R, tag="bigA")
            x_Trev = big.tile([128, TOK], F32R, tag="bigB")
            with tc.tile_pool(name="xr", bufs=24) as xrp:
                x_rows = []
                for g in range(NBLK):
                    xr = xrp.tile([128, 128], F32, tag="xr")
                    nc.gpsimd.indirect_dma_start(
                        out=xr[:],
                        out_offset=None,
                        in_=embed[:],
                        in_offset=IndirectOffsetOnAxis(
                            ap=idx_sb[:, g:g + 1], axis=0),
                    )
                    x_rows.append(xr)

                # ---------- P1: transpose (fwd + time-reversed) ----------
                with tc.tile_pool(name="xrel", bufs=4) as xrelp:
                    psts = [pq1, pq2]
                    for g in range(NBLK):
                        b_seq, tb4 = g // 4, g % 4
                        grev = b_seq * 4 + (3 - tb4)
                        xrel = xrelp.tile([128, 128], F32, tag="xrel")
                        nc.vector.tensor_tensor(
                            out=xrel[:], in0=x_rows[g][:], in1=x_rows[g][:],
                            op=Alu.max)
                        pst = psts[g % 2]
                        nc.tensor.transpose(
                            out=pst[:, 0:128], in_=xrel[:], identity=ident[:])
                        nc.vector.tensor_copy(
                            out=x_T[:, g * 128:(g + 1) * 128],
                            in_=pst[:, 0:128])
                        nc.tensor.transpose(
                            out=pst[:, 128:256], in_=xrel[:],
                            identity=rev_sb[:])
                        nc.vector.tensor_copy(
                            out=x_Trev[:, grev * 128:(grev + 1) * 128],
                            in_=pst[:, 128:256])

            # ---------- P2: bulk xproj (fp32r, N=512) ----------
            ps2s = [pw1[:], pw2[:], pw3[:]]
            n2 = 0
            for d in range(2):
                src_T = x_T if d == 0 else x_Trev
                for g in range(G4):
                    lhsT = wihr_sb[:, (d * G4 + g) * 128:(d * G4 + g + 1) * 128]
                    for b in range(Bc):
                        ps2 = ps2s[n2 % 3]
                        n2 += 1
                        nc.tensor.matmul(
                            out=ps2, lhsT=lhsT,
                            rhs=src_T[:, b * T:(b + 1) * T],
                            start=True, stop=True)
                        xp_sb = sm.tile([128, 512], F32, tag="xp_out")
                        nc.vector.tensor_scalar(
                            out=xp_sb[:], in0=ps2,
                            scalar1=bias_sb[:, d * G4 + g:d * G4 + g + 1],
                            scalar2=None, op0=Alu.add)
                        # store at PSUM block position (i,f,o,g order)
                        nc.sync.dma_start(
                            out=xproj_dram[d, GOFF[g] // 16, b],
                            in_=xp_sb[:])

            # ---------- P3: LSTM ----------
            h_f = big.tile([128, TOK], F32R, tag="bigA")
            h_b = big.tile([128, TOK], F32R, tag="bigB")
            h0 = cst.tile([128, Bc], F32)
            nc.vector.memset(h0[:], 0.0)
            c_st = cst.tile([128, 2 * Bc], F32)
            nc.vector.memset(c_st[:], 0.0)

            with tc.tile_pool(name="xpp", bufs=2) as xpp:
                for r in range(T):
                    tf, tb = r, T - 1 - r
                    c = r // CH
                    if r % CH == 0:
                        # one fused chunk: [128, (s, d, gslot, b)] — both
                        # dirs share the s index (bwd xproj is s-ordered)
                        # layout [128, (d, gslot, b, s)]: matches psum column
                        # order (d,g,b) when sliced at fixed s
                        xt = xpp.tile([128, CH * 128], F32, tag="xpc")
                        for d in range(2):
                            dst = xt[:].rearrange(
                                "p (y s) -> p y s",
                                s=CH)[:, d * 64:(d + 1) * 64]
                            src = xproj_dram[:].rearrange(
                                "e g b p t -> p (e g b) t")[
                                :, d * 64:(d + 1) * 64,
                                c * CH:(c + 1) * CH]
                            nc.sync.dma_start(out=dst, in_=src)

                    ps3 = [pq1, pq2][r % 2]
                    for d in range(2):
                        if r == 0:
                            hprev = h0[:]
                        elif d == 0:
                            hprev = f32(h_f[:, tf - 1::T])
                        else:
                            hprev = f32(h_b[:, tb + 1::T])
                        for g in range(G4):
                            lhsT = whh_sb[
                                :, (d * G4 + g) * 128:(d * G4 + g + 1) * 128]
                            nc.tensor.matmul(
                                out=ps3[:, d * 64 + GOFF[g]:
                                        d * 64 + GOFF[g] + Bc],
                                lhsT=lhsT, rhs=hprev, start=True, stop=True)
                    gsb = sm.tile([128, 128], F32, tag="gates")
                    xsl = xt[:].rearrange(
                        "p (y s) -> p y s", s=CH)[:, :, r % CH]
                    nc.vector.tensor_tensor(
                        out=gsb[:], in0=ps3[:, 0:128], in1=xsl, op=Alu.add)
                    sig = sm.tile([128, 96], F32, tag="sig")
                    nc.scalar.activation(
                        out=sig[:].rearrange(
                            "p (q d b) -> p d q b", q=3, d=2, b=Bc),
                        in_=gsb[:].rearrange("p (d x) -> p d x", d=2)[:, :, 0:48],
                        func=Act.Sigmoid)
                    tg = sm.tile([128, 2 * Bc], F32, tag="tg")
                    nc.scalar.activation(
                        out=tg[:],
                        in_=gsb[:].rearrange("p (d x) -> p d x", d=2)[:, :, 48:64],
                        func=Act.Tanh)
                    t1 = sm.tile([128, 2 * Bc], F32, tag="t1")
                    nc.gpsimd.tensor_tensor(
                        out=t1[:], in0=sig[:, 0:2 * Bc], in1=tg[:], op=Alu.mult)
                    t2 = sm.tile([128, 2 * Bc], F32, tag="t2")
                    nc.vector.tensor_tensor(
                        out=t2[:], in0=sig[:, 2 * Bc:4 * Bc], in1=c_st[:],
                        op=Alu.mult)
                    nc.vector.tensor_tensor(
                        out=c_st[:], in0=t1[:], in1=t2[:], op=Alu.add)
                    tcx = sm.tile([128, 2 * Bc], F32, tag="tc")
                    nc.scalar.activation(out=tcx[:], in_=c_st[:], func=Act.Tanh)
                    mt = sm.tile([128, Bc], F32, tag="mt")
                    nc.gpsimd.tensor_scalar(
                        out=mt[:], in0=lens_sb[:], scalar1=float(tb),
                        scalar2=None, op0=Alu.is_gt)
                    nc.vector.tensor_tensor(
                        out=h_f[:, tf::T], in0=sig[:, 4 * Bc:5 * Bc],
                        in1=tcx[:, 0:Bc], op=Alu.mult)
                    hbt = sm.tile([128, Bc], F32, tag="hbt")
                    nc.gpsimd.tensor_tensor(
                        out=hbt[:], in0=sig[:, 5 * Bc:6 * Bc],
                        in1=tcx[:, Bc:2 * Bc], op=Alu.mult)
                    nc.vector.tensor_tensor(
                        out=h_b[:, tb::T], in0=hbt[:], in1=mt[:], op=Alu.mult)
                    nc.gpsimd.tensor_tensor(
                        out=c_st[:, Bc:2 * Bc], in0=c_st[:, Bc:2 * Bc],
                        in1=mt[:], op=Alu.mult)

            # ---------- P4: emissions em^T = [9, TOK] (fp32r, N=512) ----------
            for blk in range(Bc):
                ps4 = [pw1, pw2][blk % 2][0:K, :]
                sl = slice(blk * T, (blk + 1) * T)
                nc.tensor.matmul(
                    out=ps4, lhsT=woutTr_sb[:, 0:K], rhs=h_f[:, sl],
                    start=True, stop=False)
                nc.tensor.matmul(
                    out=ps4, lhsT=woutTr_sb[:, K:2 * K], rhs=h_b[:, sl],
                    start=False, stop=True)
                em_sb = sm.tile([K, T], F32, tag="em_sb")
                nc.vector.tensor_scalar(
                    out=em_sb[:], in0=ps4, scalar1=bout9_sb[:, 0:1],
                    scalar2=None, op0=Alu.add)
                nc.sync.dma_start(out=emT_dram[:, sl], in_=em_sb[:])

            # ---------- P5: bidirectional Viterbi DP ----------
            # em1[b, (k, t)] = em[b*T + t, k]; em2 same shifted by one t.
            # em1 reuses the bigB slot (h_b is dead after P4).
            dpbig_cm = tc.tile_pool(name="dpbig", bufs=1)
            dbig = dpbig_cm.__enter__()
            em1 = big.tile([Bc, K * T], F32, tag="bigB")
            em2 = dbig.tile([Bc, K * T], F32, tag="em2")
            src1 = emT_dram[:].rearrange("k (b t) -> b k t", b=Bc)
            nc.sync.dma_start(
                out=em1[:].rearrange("b (k t) -> b k t", k=K), in_=src1)
            # em2[b, k, s] = em[b, s+1, k]: per-b loads (row misalignment
            # across the b boundary prevents a single grouped DMA)
            for b in range(Bc):
                nc.sync.dma_start(
                    out=em2[b:b + 1].rearrange(
                        "b (k t) -> b k t", k=K)[:, :, 0:T - 1],
                    in_=emT_dram[:, b * T + 1:(b + 1) * T].unsqueeze(0))

            M_hist = dbig.tile([Bc, T * K], F32, tag="Mh")
            B_hist = dbig.tile([Bc, T * K], F32, tag="Bh")
            nc.vector.tensor_copy(out=M_hist[:, 0:K], in_=start9_sb[:])
            nc.vector.tensor_copy(
                out=B_hist[:, (T - 1) * K:T * K], in_=end9_sb[:])

            # slab chunks: fwd B_f[s][i,j] = transT9[i,j] + em[s][j]
            #   (B_f[s] drives step M_s -> M_{s+1}; no masking needed, the
            #    post-length region is never read)
            # bwd B_b[s][i,j] = mask[s+1] ? trans[i,j] + em[s+1][j]
            #                             : maxplus-identity (0 diag/-1e9 off)
            #   computed as m*(trans+em) + invm*identM — both products are
            #   exactly 0 or the exact value, so no 1e9 cancellation error.
            nsteps = T - 1  # 511
            chunks = [(s0, min(DPCH, nsteps - s0))
                      for s0 in range(0, nsteps, DPCH)]

            def em_view(emt, s0, ns):
                # [16, ns, 9i, 9j] reading em[s0+s][j]: strides s:1,i:0,j:T
                v = emt[:].rearrange("b (k t) -> b t k", k=K)
                v = v[:, s0:s0 + ns]                       # [16, ns, 9j]
                return v.unsqueeze(2).to_broadcast([Bc, ns, K, K])

            def c81(cst_ap, ns):
                # [16, 81] const -> [16, ns, 9, 9]
                v = cst_ap.rearrange("b (i j) -> b i j", i=K)
                return v.unsqueeze(1).to_broadcast([Bc, ns, K, K])

            def mview(src, s0, ns):
                v = src[:, s0 + 1:s0 + 1 + ns]
                return v.unsqueeze(2).unsqueeze(3).to_broadcast(
                    [Bc, ns, K, K])

            def sview(t, ns):
                return t[:].rearrange("b (s i j) -> b s i j",
                                      s=DPCH, i=K)[:, 0:ns]

            with tc.tile_pool(name="dp", bufs=2) as dpp:
                def build_f(s0, ns):
                    sf = dpp.tile([Bc, DPCH * 81], F32, tag="slabf")
                    nc.gpsimd.tensor_tensor(
                        out=sview(sf, ns), in0=em_view(em1, s0, ns),
                        in1=c81(transT9_sb[:], ns), op=Alu.add)
                    return sf

                def build_b(s0, ns):
                    # tmp reuses the slabf tag: the fwd sweep is fully done
                    # before any build_b's ops are scheduled on gpsimd/DVE
                    sb_ = dpp.tile([Bc, DPCH * 81], F32, tag="slabb")
                    tmp = dpp.tile([Bc, DPCH * 81], F32, tag="slabf")
                    nc.gpsimd.tensor_tensor(
                        out=sview(tmp, ns), in0=em_view(em2, s0, ns),
                        in1=c81(trans9_sb[:], ns), op=Alu.add)
                    nc.gpsimd.tensor_tensor(
                        out=sview(tmp, ns), in0=sview(tmp, ns),
                        in1=mview(mask_sb, s0, ns), op=Alu.mult)
                    nc.gpsimd.tensor_tensor(
                        out=sview(sb_, ns), in0=c81(identM9_sb[:], ns),
                        in1=mview(invm_sb, s0, ns), op=Alu.mult)
                    nc.gpsimd.tensor_tensor(
                        out=sview(sb_, ns), in0=sview(sb_, ns),
                        in1=sview(tmp, ns), op=Alu.add)
                    return sb_

                cand_f = sm.tile([Bc, 81], F32, tag="cand_f")
                cand_b = sm.tile([Bc, 81], F32, tag="cand_b")

                # forward sweep (ascending)
                pend = build_f(*chunks[0])
                for ci, (s0, ns) in enumerate(chunks):
                    sf = pend
                    if ci + 1 < len(chunks):
                        pend = build_f(*chunks[ci + 1])
                    for k in range(ns):
                        s = s0 + k   # M_{s+1} from M_s and B_f[s]
                        nc.vector.tensor_tensor(
                            out=cand_f[:].rearrange("b (i j) -> b i j", i=K),
                            in0=M_hist[:, s * K:(s + 1) * K].unsqueeze(1)
                                .to_broadcast([Bc, K, K]),
                            in1=sview(sf, ns)[:, k], op=Alu.add)
                        nc.vector.tensor_reduce(
                            out=M_hist[:, (s + 1) * K:(s + 2) * K],
                            in_=cand_f[:].rearrange("b (i j) -> b i j", i=K),
                            axis=AxX, op=Alu.max)

                # backward sweep (descending)
                pend = build_b(*chunks[-1])
                for ci in range(len(chunks) - 1, -1, -1):
                    s0, ns = chunks[ci]
                    sb_ = pend
                    if ci > 0:
                        pend = build_b(*chunks[ci - 1])
                    for k in range(ns - 1, -1, -1):
                        s = s0 + k   # beta_s from beta_{s+1} and B_b[s]
                        nc.vector.tensor_tensor(
                            out=cand_b[:].rearrange("b (i j) -> b i j", i=K),
                            in0=B_hist[:, (s + 1) * K:(s + 2) * K].unsqueeze(1)
                                .to_broadcast([Bc, K, K]),
                            in1=sview(sb_, ns)[:, k], op=Alu.add)
                        nc.vector.tensor_reduce(
                            out=B_hist[:, s * K:(s + 1) * K],
                            in_=cand_b[:].rearrange("b (i j) -> b i j", i=K),
                            axis=AxX, op=Alu.max)

            # ---------- P6: tags = argmax_i(M + em + beta) ----------
            tags_f = dbig.tile([Bc, T], F32, tag="tags_f")
            # sc aliases em2's allocation (em2 is dead after the bwd slab
            # builds); eq aliases the bigA slot (h_f is dead after P4).
            sc = dbig.tile([Bc, T * K], F32, tag="em2")
            nc.vector.tensor_tensor(
                out=sc[:], in0=M_hist[:], in1=B_hist[:], op=Alu.add)
            emv = em1[:].rearrange("b (k t) -> b t k", k=K)
            nc.vector.tensor_tensor(
                out=sc[:].rearrange("b (t k) -> b t k", k=K),
                in0=sc[:].rearrange("b (t k) -> b t k", k=K),
                in1=emv, op=Alu.add)
            mx = dbig.tile([Bc, T], F32, tag="mx")
            nc.vector.tensor_reduce(
                out=mx[:], in_=sc[:].rearrange("b (t k) -> b t k", k=K),
                axis=AxX, op=Alu.max)
            eq = big.tile([Bc, T * K], F32, tag="bigA")
            nc.vector.tensor_tensor(
                out=eq[:].rearrange("b (t k) -> b t k", k=K),
                in0=sc[:].rearrange("b (t k) -> b t k", k=K),
                in1=mx[:].unsqueeze(2).to_broadcast([Bc, T, K]),
                op=Alu.is_equal)
            irev_v = irev9_sb[:].unsqueeze(1).to_broadcast([Bc, T, K])
            nc.vector.tensor_tensor(
                out=eq[:].rearrange("b (t k) -> b t k", k=K),
                in0=eq[:].rearrange("b (t k) -> b t k", k=K),
                in1=irev_v, op=Alu.mult)
            nc.vector.tensor_reduce(
                out=tags_f[:], in_=eq[:].rearrange("b (t k) -> b t k", k=K),
                axis=AxX, op=Alu.max)
            nc.vector.tensor_scalar(
                out=tags_f[:], in0=tags_f[:], scalar1=-1.0, scalar2=8.0,
                op0=Alu.mult, op1=Alu.add)
            nc.vector.tensor_tensor(
                out=tags_f[:], in0=tags_f[:], in1=mask_sb[:], op=Alu.mult)
            tags_i = dbig.tile([Bc, T], I32, tag="tags_i")
            nc.vector.tensor_copy(out=tags_i[:], in_=tags_f[:])
            nc.sync.dma_start(out=tags_out[:], in_=tags_i[:])
            dpbig_cm.__exit__(None, None, None)
            psp_cm.__exit__(None, None, None)

    nc.finalize()
    return nc


_NC_CACHE = None


def _get_program():
    global _NC_CACHE
    if _NC_CACHE is None:
        _NC_CACHE = build_program()
    return _NC_CACHE


def make_in_maps(sentences, lengths, embed, Wih_f, Whh_f, bih_f, bhh_f,
                 Wih_b, Whh_b, bih_b, bhh_b, W_out, b_out, start_t, end_t,
                 trans):
    sentences = np.ascontiguousarray(sentences, dtype=np.int32)
    embed = np.ascontiguousarray(embed, dtype=np.float32)
    lengths = np.asarray(lengths)

    whh_pack = np.zeros((128, 1024), np.float32)
    wih_pack = np.zeros((128, 1024), np.float32)
    bias_pack = np.zeros((128, 8), np.float32)
    for d, (Wih, Whh, bi, bh) in enumerate(
            ((Wih_f, Whh_f, bih_f, bhh_f), (Wih_b, Whh_b, bih_b, bhh_b))):
        for g in range(G4):
            whh_pack[:, (d * G4 + g) * 128:(d * G4 + g + 1) * 128] = \
                np.asarray(Whh)[g * 128:(g + 1) * 128, :].T
            wih_pack[:, (d * G4 + g) * 128:(d * G4 + g + 1) * 128] = \
                np.asarray(Wih)[g * 128:(g + 1) * 128, :].T
            bias_pack[:, d * G4 + g] = \
                (np.asarray(bi) + np.asarray(bh))[g * 128:(g + 1) * 128]

    W_out = np.asarray(W_out, np.float32)
    woutT = np.zeros((128, 18), np.float32)
    woutT[:, 0:K] = W_out[:, :128].T
    woutT[:, K:2 * K] = W_out[:, 128:].T
    bout9 = np.asarray(b_out, np.float32)[:, None].copy()

    rev_id = np.zeros((128, 128), np.float32)
    rev_id[np.arange(128), 127 - np.arange(128)] = 1.0

    trans_np = np.asarray(trans, np.float32)
    identM = np.full((K, K), NEG, np.float32)
    np.fill_diagonal(identM, 0.0)
    transT9 = np.broadcast_to(trans_np.T.reshape(-1)[None], (Bc, 81)).copy()
    trans9 = np.broadcast_to(trans_np.reshape(-1)[None], (Bc, 81)).copy()
    identM9 = np.broadcast_to(identM.reshape(-1)[None], (Bc, 81)).copy()

    start9 = np.broadcast_to(
        np.asarray(start_t, np.float32)[None, :], (Bc, K)).copy()
    end9 = np.broadcast_to(
        np.asarray(end_t, np.float32)[None, :], (Bc, K)).copy()
    ii = np.arange(K, dtype=np.float32)
    irev9 = np.broadcast_to((8.0 - ii)[None, :], (Bc, K)).copy()
    tt = np.arange(T)

    in_maps = []
    for c in range(NC):
        sl = slice(c * Bc, (c + 1) * Bc)
        sents_c = sentences[sl]
        lens_c = np.asarray(lengths[sl], np.float32)
        idx_np = np.zeros((128, NBLK), np.int32)
        p = np.arange(128)
        for g in range(NBLK):
            bt = g * 128 + p
            idx_np[:, g] = sents_c[bt // T, bt % T]
        lens_bc = np.broadcast_to(lens_c[None, :], (128, Bc)).copy()
        mask_np = (tt[None, :] < lens_c[:, None]).astype(np.float32)
        in_maps.append({
            "embed": embed,
            "idx": idx_np,
            "whh_pack": whh_pack, "wih_pack": wih_pack, "bias_pack": bias_pack,
            "lens_b": lens_bc,
            "woutT": woutT, "bout9": bout9, "rev_id": rev_id,
            "transT9": transT9, "trans9": trans9, "identM9": identM9,
            "start9": start9, "end9": end9, "irev9": irev9,
            "mask_dp": mask_np, "invm_dp": 1.0 - mask_np,
        })
    return in_maps


def run(inputs, trace=False, **kw):
    nc = _get_program()
    in_maps = make_in_maps(**inputs)
    res = run_bass_kernel_spmd(nc, in_maps, list(range(NC)), trace=trace, **kw)
    tags = np.concatenate([r["tags"] for r in res.results], axis=0)
    return tags.astype(np.int32), res


def kernel(**inputs):
    tags, _ = run(inputs)
    return tags


# revision 40
# speedup vs baseline: 1.5890x; 1.5478x over previous
"""BiLSTM-CRF Viterbi decode on 8 Trainium2 NeuronCores.

Data-parallel over batch: each core handles 16 of 128 sequences.

Per-core phases:
  P0 embedding gather (indirect DMA, 128 rows per DMA)
  P1 PE-transpose x_rows [tok,E] -> x_T [E,tok] and x_Trev (time-reversed
     per sequence, via anti-diagonal identity)
  P2 bulk input projection xproj = Wih_g @ x_T (+bias) staged to DRAM,
     fp32r matmuls (N=512); bwd direction projected from x_Trev so its
     DRAM layout is s-ordered (s = T-1-t)
  P3 512 fused fwd+bwd LSTM rounds (gate-dim on partitions, [128,16]
     tiles); one [128,128] xproj+psum add per round; ping-pong PSUM
  P4 emissions em^T = W_out @ h as [9,512] fp32r matmuls staged to DRAM
  P5 forward (M-form) and backward (beta) Viterbi DPs on DVE, reading
     precombined slabs B[s][i,j] = trans +/- em built on GpSimd; masked
     steps become max-plus identity so no per-step predication
  P6 tags = argmax_i(M_t + em_t + beta_t), bulk DVE ops

All matmuls fp32/fp32r (bf16 flips ~50 tags vs the fp32 reference).
"""

import ml_dtypes
import numpy as np

import concourse.bacc as bacc
import concourse.bass as bass
import concourse.mybir as mybir
import concourse.tile as tile
from concourse.bass import IndirectOffsetOnAxis
from concourse.bass_utils import run_bass_kernel_spmd
from concourse.masks import make_identity

F32 = mybir.dt.float32
F32R = mybir.dt.float32r
BF16 = mybir.dt.bfloat16
I32 = mybir.dt.int32
REC_BF16 = True       # bf16 LSTM recurrence (1-pass matmuls, FWL ldweights)
Alu = mybir.AluOpType
Act = mybir.ActivationFunctionType
AxX = mybir.AxisListType.X

B, T, V, E, H, K = 128, 512, 100000, 128, 128, 9
NC = 8
Bc = B // NC          # 16 sequences per core
TOK = Bc * T          # 8192 tokens per core, flat index bt = b*T + t (b-major)
NBLK = TOK // 128     # 64 gather/transpose blocks
G4 = 4
# gate order in weights: i, f, g, o (torch). psum cols per dir: i(0) f(16) o(32) g(48)
GOFF = {0: 0, 1: 16, 3: 32, 2: 48}
CH = 32               # LSTM rounds per xproj chunk
NCH = T // CH
DPCH = 32             # Viterbi DP steps per slab chunk
NEG = -1.0e9


def f32(ap):
    return ap.bitcast(F32)


def build_program():
    nc = bacc.Bacc(None, target_bir_lowering=False)

    # ---------------- dram parameters ----------------
    embed = nc.declare_dram_parameter("embed", [V, E], F32, isOutput=False)
    idx = nc.declare_dram_parameter("idx", [128, NBLK], I32, isOutput=False)
    RDT = BF16 if REC_BF16 else F32
    whh_pack = nc.declare_dram_parameter("whh_pack", [128, 1024], RDT, isOutput=False)
    wih_pack = nc.declare_dram_parameter("wih_pack", [128, 1024], F32, isOutput=False)
    bias_pack = nc.declare_dram_parameter("bias_pack", [128, 8], F32, isOutput=False)
    lens_b = nc.declare_dram_parameter("lens_b", [128, Bc], F32, isOutput=False)
    woutT = nc.declare_dram_parameter("woutT", [128, 18], F32, isOutput=False)
    bout9 = nc.declare_dram_parameter("bout9", [K, 1], F32, isOutput=False)
    rev_id = nc.declare_dram_parameter("rev_id", [128, 128], F32, isOutput=False)
    transT9 = nc.declare_dram_parameter("transT9", [Bc, 81], F32, isOutput=False)
    trans9 = nc.declare_dram_parameter("trans9", [Bc, 81], F32, isOutput=False)
    identM9 = nc.declare_dram_parameter("identM9", [Bc, 81], F32, isOutput=False)
    start9 = nc.declare_dram_parameter("start9", [Bc, K], F32, isOutput=False)
    end9 = nc.declare_dram_parameter("end9", [Bc, K], F32, isOutput=False)
    irev9_p = nc.declare_dram_parameter("irev9", [Bc, K], F32, isOutput=False)
    mask_dp = nc.declare_dram_parameter("mask_dp", [Bc, T], F32, isOutput=False)
    invm_dp = nc.declare_dram_parameter("invm_dp", [Bc, T], F32, isOutput=False)
    tags_out = nc.declare_dram_parameter("tags", [Bc, T], I32, isOutput=True)

    # ---------------- dram internals ----------------
    xproj_dram = nc.dram_tensor("xproj_dram", [2, G4, Bc, 128, T], F32)
    emT_dram = nc.dram_tensor("emT_dram", [K, TOK], F32)

    with tile.TileContext(nc) as tc:
        with (
            tc.tile_pool(name="big", bufs=1) as big,
            tc.tile_pool(name="consts", bufs=1) as cst,
            tc.tile_pool(name="small", bufs=3) as sm,
        ):
            # ---------- constants ----------
            idx_sb = cst.tile([128, NBLK], I32)
            nc.sync.dma_start(out=idx_sb[:], in_=idx[:])
            whh_sb = cst.tile([128, 1024], RDT)
            nc.sync.dma_start(out=whh_sb[:], in_=whh_pack[:])
            wih_sb = cst.tile([128, 1024], F32)
            nc.sync.dma_start(out=wih_sb[:], in_=wih_pack[:])
            bias_sb = cst.tile([128, 8], F32)
            nc.sync.dma_start(out=bias_sb[:], in_=bias_pack[:])
            lens_sb = cst.tile([128, Bc], F32)
            nc.sync.dma_start(out=lens_sb[:], in_=lens_b[:])
            woutT_sb = cst.tile([128, 18], F32)
            nc.sync.dma_start(out=woutT_sb[:], in_=woutT[:])
            # device-rounded fp32r copies (the fp32r matmul path requires
            # its operands to be produced pre-rounded to fp32r)
            wihr_sb = cst.tile([128, 1024], F32R)
            nc.vector.tensor_copy(out=wihr_sb[:], in_=wih_sb[:])
            woutTr_sb = cst.tile([128, 18], BF16 if REC_BF16 else F32R)
            nc.vector.tensor_copy(out=woutTr_sb[:], in_=woutT_sb[:])
            bout9_sb = cst.tile([K, 1], F32)
            nc.sync.dma_start(out=bout9_sb[:], in_=bout9[:])
            rev_dma = cst.tile([128, 128], F32)
            nc.sync.dma_start(out=rev_dma[:], in_=rev_id[:])
            rev_sb = cst.tile([128, 128], F32)
            nc.vector.tensor_copy(out=rev_sb[:], in_=rev_dma[:])
            ident = cst.tile([128, 128], F32)
            make_identity(nc, ident[:])

            # DP constants (p0-15)
            transT9_sb = cst.tile([Bc, 81], F32)
            nc.sync.dma_start(out=transT9_sb[:], in_=transT9[:])
            trans9_sb = cst.tile([Bc, 81], F32)
            nc.sync.dma_start(out=trans9_sb[:], in_=trans9[:])
            identM9_sb = cst.tile([Bc, 81], F32)
            nc.sync.dma_start(out=identM9_sb[:], in_=identM9[:])
            start9_sb = cst.tile([Bc, K], F32)
            nc.sync.dma_start(out=start9_sb[:], in_=start9[:])
            end9_sb = cst.tile([Bc, K], F32)
            nc.sync.dma_start(out=end9_sb[:], in_=end9[:])
            irev9_sb = cst.tile([Bc, K], F32)
            nc.sync.dma_start(out=irev9_sb[:], in_=irev9_p[:])
            mask_sb = cst.tile([Bc, T], F32)
            nc.sync.dma_start(out=mask_sb[:], in_=mask_dp[:])
            invm_sb = cst.tile([Bc, T], F32)
            nc.sync.dma_start(out=invm_sb[:], in_=invm_dp[:])

            # PE "absorber" ops: self-loading (fp32/fp32r) matmuls may carry
            # at most ONE sync wait in walrus codegen. These tiny ops advance
            # PE's vector clock over one-time deps (identity from Pool,
            # const-weight DMA lanes) so real matmuls each need <=1 wait.
            psp_cm = tc.tile_pool(name="psglob", bufs=1, space="PSUM")
            psp = psp_cm.__enter__()
            pq1 = psp.tile([128, 512], F32, tag="pq1", name="pq1")
            pq2 = psp.tile([128, 512], F32, tag="pq2", name="pq2")
            pw1 = psp.tile([128, 512], F32, tag="pw1", name="pw1")
            pw2 = psp.tile([128, 512], F32, tag="pw2", name="pw2")
            pw3 = psp.tile([128, 512], F32, tag="pw3", name="pw3")
            nc.tensor.transpose(out=pq1[:, 0:128], in_=ident[:], identity=ident[:])
            nc.tensor.transpose(out=pq2[:, 0:128], in_=rev_sb[:], identity=ident[:])
            nc.tensor.matmul(out=pq2[0:1, 0:1], lhsT=whh_sb[:, 0:1],
                             rhs=whh_sb[:, 0:1], start=True, stop=True)

            # ---------- P0: gather ----------
            x_T = big.tile([128, TOK], F32R, tag="bigA")
            x_Trev = big.tile([128, TOK], F32R, tag="bigB")
            with tc.tile_pool(name="xr", bufs=24) as xrp:
                x_rows = []
                for g in range(NBLK):
                    xr = xrp.tile([128, 128], F32, tag="xr")
                    nc.gpsimd.indirect_dma_start(
                        out=xr[:],
                        out_offset=None,
                        in_=embed[:],
                        in_offset=IndirectOffsetOnAxis(
                            ap=idx_sb[:, g:g + 1], axis=0),
                    )
                    x_rows.append(xr)

                # ---------- P1: transpose (fwd + time-reversed) ----------
                with tc.tile_pool(name="xrel", bufs=4) as xrelp:
                    psts = [pq1, pq2]
                    for g in range(NBLK):
                        b_seq, tb4 = g // 4, g % 4
                        grev = b_seq * 4 + (3 - tb4)
                        xrel = xrelp.tile([128, 128], F32, tag="xrel")
                        nc.vector.tensor_tensor(
                            out=xrel[:], in0=x_rows[g][:], in1=x_rows[g][:],
                            op=Alu.max)
                        pst = psts[g % 2]
                        nc.tensor.transpose(
                            out=pst[:, 0:128], in_=xrel[:], identity=ident[:])
                        nc.vector.tensor_copy(
                            out=x_T[:, g * 128:(g + 1) * 128],
                            in_=pst[:, 0:128])
                        nc.tensor.transpose(
                            out=pst[:, 128:256], in_=xrel[:],
                            identity=rev_sb[:])
                        nc.vector.tensor_copy(
                            out=x_Trev[:, grev * 128:(grev + 1) * 128],
                            in_=pst[:, 128:256])

            # ---------- P2: bulk xproj (fp32r, N=512) ----------
            ps2s = [pw1[:], pw2[:], pw3[:]]
            n2 = 0
            for d in range(2):
                src_T = x_T if d == 0 else x_Trev
                for g in range(G4):
                    lhsT = wihr_sb[:, (d * G4 + g) * 128:(d * G4 + g + 1) * 128]
                    for b in range(Bc):
                        ps2 = ps2s[n2 % 3]
                        n2 += 1
                        nc.tensor.matmul(
                            out=ps2, lhsT=lhsT,
                            rhs=src_T[:, b * T:(b + 1) * T],
                            start=True, stop=True)
                        xp_sb = sm.tile([128, 512], F32, tag="xp_out")
                        nc.vector.tensor_scalar(
                            out=xp_sb[:], in0=ps2,
                            scalar1=bias_sb[:, d * G4 + g:d * G4 + g + 1],
                            scalar2=None, op0=Alu.add)
                        # store at PSUM block position (i,f,o,g order)
                        nc.sync.dma_start(
                            out=xproj_dram[d, GOFF[g] // 16, b],
                            in_=xp_sb[:])

            # ---------- P3: LSTM ----------
            HDT = BF16 if REC_BF16 else F32R
            h_f = big.tile([128, TOK], HDT, tag="bigA")
            h_b = big.tile([128, TOK], HDT, tag="bigB")
            h0 = cst.tile([128, Bc], HDT)
            nc.vector.memset(h0[:], 0.0)
            c_st = cst.tile([128, 2 * Bc], F32)
            nc.vector.memset(c_st[:], 0.0)

            with tc.tile_pool(name="xpp", bufs=2) as xpp:
                for r in range(T):
                    tf, tb = r, T - 1 - r
                    c = r // CH
                    if r % CH == 0:
                        # one fused chunk: [128, (s, d, gslot, b)] — both
                        # dirs share the s index (bwd xproj is s-ordered)
                        # layout [128, (d, gslot, b, s)]: matches psum column
                        # order (d,g,b) when sliced at fixed s
                        xt = xpp.tile([128, CH * 128], F32, tag="xpc")
                        for d in range(2):
                            dst = xt[:].rearrange(
                                "p (y s) -> p y s",
                                s=CH)[:, d * 64:(d + 1) * 64]
                            src = xproj_dram[:].rearrange(
                                "e g b p t -> p (e g b) t")[
                                :, d * 64:(d + 1) * 64,
                                c * CH:(c + 1) * CH]
                            nc.sync.dma_start(out=dst, in_=src)

                    ps3 = [pq1, pq2][r % 2]
                    for d in range(2):
                        if r == 0:
                            hprev = h0[:]
                        elif d == 0:
                            hprev = h_f[:, tf - 1::T]
                        else:
                            hprev = h_b[:, tb + 1::T]
                        if not REC_BF16:
                            hprev = f32(hprev)
                        for g in range(G4):
                            lhsT = whh_sb[
                                :, (d * G4 + g) * 128:(d * G4 + g + 1) * 128]
                            nc.tensor.matmul(
                                out=ps3[:, d * 64 + GOFF[g]:
                                        d * 64 + GOFF[g] + Bc],
                                lhsT=lhsT, rhs=hprev, start=True, stop=True)
                    gsb = sm.tile([128, 128], F32, tag="gates")
                    xsl = xt[:].rearrange(
                        "p (y s) -> p y s", s=CH)[:, :, r % CH]
                    nc.vector.tensor_tensor(
                        out=gsb[:], in0=ps3[:, 0:128], in1=xsl, op=Alu.add)
                    sig = sm.tile([128, 96], F32, tag="sig")
                    nc.scalar.activation(
                        out=sig[:].rearrange(
                            "p (q d b) -> p d q b", q=3, d=2, b=Bc),
                        in_=gsb[:].rearrange("p (d x) -> p d x", d=2)[:, :, 0:48],
                        func=Act.Sigmoid)
                    tg = sm.tile([128, 2 * Bc], F32, tag="tg")
                    nc.scalar.activation(
                        out=tg[:],
                        in_=gsb[:].rearrange("p (d x) -> p d x", d=2)[:, :, 48:64],
                        func=Act.Tanh)
                    t1 = sm.tile([128, 2 * Bc], F32, tag="t1")
                    nc.gpsimd.tensor_tensor(
                        out=t1[:], in0=sig[:, 0:2 * Bc], in1=tg[:], op=Alu.mult)
                    t2 = sm.tile([128, 2 * Bc], F32, tag="t2")
                    nc.vector.tensor_tensor(
                        out=t2[:], in0=sig[:, 2 * Bc:4 * Bc], in1=c_st[:],
                        op=Alu.mult)
                    nc.vector.tensor_tensor(
                        out=c_st[:], in0=t1[:], in1=t2[:], op=Alu.add)
                    tcx = sm.tile([128, 2 * Bc], F32, tag="tc")
                    nc.scalar.activation(out=tcx[:], in_=c_st[:], func=Act.Tanh)
                    mt = sm.tile([128, Bc], F32, tag="mt")
                    nc.gpsimd.tensor_scalar(
                        out=mt[:], in0=lens_sb[:], scalar1=float(tb),
                        scalar2=None, op0=Alu.is_gt)
                    nc.vector.tensor_tensor(
                        out=h_f[:, tf::T], in0=sig[:, 4 * Bc:5 * Bc],
                        in1=tcx[:, 0:Bc], op=Alu.mult)
                    hbt = sm.tile([128, Bc], F32, tag="hbt")
                    nc.gpsimd.tensor_tensor(
                        out=hbt[:], in0=sig[:, 5 * Bc:6 * Bc],
                        in1=tcx[:, Bc:2 * Bc], op=Alu.mult)
                    nc.vector.tensor_tensor(
                        out=h_b[:, tb::T], in0=hbt[:], in1=mt[:], op=Alu.mult)
                    nc.gpsimd.tensor_tensor(
                        out=c_st[:, Bc:2 * Bc], in0=c_st[:, Bc:2 * Bc],
                        in1=mt[:], op=Alu.mult)

            # ---------- P4: emissions em^T = [9, TOK] (fp32r, N=512) ----------
            for blk in range(Bc):
                ps4 = [pw1, pw2][blk % 2][0:K, :]
                sl = slice(blk * T, (blk + 1) * T)
                nc.tensor.matmul(
                    out=ps4, lhsT=woutTr_sb[:, 0:K], rhs=h_f[:, sl],
                    start=True, stop=False)
                nc.tensor.matmul(
                    out=ps4, lhsT=woutTr_sb[:, K:2 * K], rhs=h_b[:, sl],
                    start=False, stop=True)
                em_sb = sm.tile([K, T], F32, tag="em_sb")
                nc.vector.tensor_scalar(
                    out=em_sb[:], in0=ps4, scalar1=bout9_sb[:, 0:1],
                    scalar2=None, op0=Alu.add)
                nc.sync.dma_start(out=emT_dram[:, sl], in_=em_sb[:])

            # ---------- P5: bidirectional Viterbi DP ----------
            # em1[b, (k, t)] = em[b*T + t, k]; em2 same shifted by one t.
            # em1 reuses the bigB slot (h_b is dead after P4).
            dpbig_cm = tc.tile_pool(name="dpbig", bufs=1)
            dbig = dpbig_cm.__enter__()
            em1 = big.tile([Bc, K * T], F32, tag="bigB")
            em2 = dbig.tile([Bc, K * T], F32, tag="em2")
            src1 = emT_dram[:].rearrange("k (b t) -> b k t", b=Bc)
            nc.sync.dma_start(
                out=em1[:].rearrange("b (k t) -> b k t", k=K), in_=src1)
            # em2[b, k, s] = em[b, s+1, k]: per-b loads (row misalignment
            # across the b boundary prevents a single grouped DMA)
            for b in range(Bc):
                nc.sync.dma_start(
                    out=em2[b:b + 1].rearrange(
                        "b (k t) -> b k t", k=K)[:, :, 0:T - 1],
                    in_=emT_dram[:, b * T + 1:(b + 1) * T].unsqueeze(0))

            M_hist = dbig.tile([Bc, T * K], F32, tag="Mh")
            B_hist = dbig.tile([Bc, T * K], F32, tag="Bh")
            nc.vector.tensor_copy(out=M_hist[:, 0:K], in_=start9_sb[:])
            nc.vector.tensor_copy(
                out=B_hist[:, (T - 1) * K:T * K], in_=end9_sb[:])

            # slab chunks: fwd B_f[s][i,j] = transT9[i,j] + em[s][j]
            #   (B_f[s] drives step M_s -> M_{s+1}; no masking needed, the
            #    post-length region is never read)
            # bwd B_b[s][i,j] = mask[s+1] ? trans[i,j] + em[s+1][j]
            #                             : maxplus-identity (0 diag/-1e9 off)
            #   computed as m*(trans+em) + invm*identM — both products are
            #   exactly 0 or the exact value, so no 1e9 cancellation error.
            nsteps = T - 1  # 511
            chunks = [(s0, min(DPCH, nsteps - s0))
                      for s0 in range(0, nsteps, DPCH)]

            def em_view(emt, s0, ns):
                # [16, ns, 9i, 9j] reading em[s0+s][j]: strides s:1,i:0,j:T
                v = emt[:].rearrange("b (k t) -> b t k", k=K)
                v = v[:, s0:s0 + ns]                       # [16, ns, 9j]
                return v.unsqueeze(2).to_broadcast([Bc, ns, K, K])

            def c81(cst_ap, ns):
                # [16, 81] const -> [16, ns, 9, 9]
                v = cst_ap.rearrange("b (i j) -> b i j", i=K)
                return v.unsqueeze(1).to_broadcast([Bc, ns, K, K])

            def mview(src, s0, ns):
                v = src[:, s0 + 1:s0 + 1 + ns]
                return v.unsqueeze(2).unsqueeze(3).to_broadcast(
                    [Bc, ns, K, K])

            def sview(t, ns):
                return t[:].rearrange("b (s i j) -> b s i j",
                                      s=DPCH, i=K)[:, 0:ns]

            with tc.tile_pool(name="dp", bufs=2) as dpp:
                def build_f(s0, ns):
                    sf = dpp.tile([Bc, DPCH * 81], F32, tag="slabf")
                    nc.gpsimd.tensor_tensor(
                        out=sview(sf, ns), in0=em_view(em1, s0, ns),
                        in1=c81(transT9_sb[:], ns), op=Alu.add)
                    return sf

                def build_b(s0, ns):
                    # tmp reuses the slabf tag: the fwd sweep is fully done
                    # before any build_b's ops are scheduled on gpsimd/DVE
                    sb_ = dpp.tile([Bc, DPCH * 81], F32, tag="slabb")
                    tmp = dpp.tile([Bc, DPCH * 81], F32, tag="slabf")
                    nc.gpsimd.tensor_tensor(
                        out=sview(tmp, ns), in0=em_view(em2, s0, ns),
                        in1=c81(trans9_sb[:], ns), op=Alu.add)
                    nc.gpsimd.tensor_tensor(
                        out=sview(tmp, ns), in0=sview(tmp, ns),
                        in1=mview(mask_sb, s0, ns), op=Alu.mult)
                    nc.gpsimd.tensor_tensor(
                        out=sview(sb_, ns), in0=c81(identM9_sb[:], ns),
                        in1=mview(invm_sb, s0, ns), op=Alu.mult)
                    nc.gpsimd.tensor_tensor(
                        out=sview(sb_, ns), in0=sview(sb_, ns),
                        in1=sview(tmp, ns), op=Alu.add)
                    return sb_

                cand_f = sm.tile([Bc, 81], F32, tag="cand_f")
                cand_b = sm.tile([Bc, 81], F32, tag="cand_b")

                # forward sweep (ascending)
                pend = build_f(*chunks[0])
                for ci, (s0, ns) in enumerate(chunks):
                    sf = pend
                    if ci + 1 < len(chunks):
                        pend = build_f(*chunks[ci + 1])
                    for k in range(ns):
                        s = s0 + k   # M_{s+1} from M_s and B_f[s]
                        nc.vector.tensor_tensor(
                            out=cand_f[:].rearrange("b (i j) -> b i j", i=K),
                            in0=M_hist[:, s * K:(s + 1) * K].unsqueeze(1)
                                .to_broadcast([Bc, K, K]),
                            in1=sview(sf, ns)[:, k], op=Alu.add)
                        nc.vector.tensor_reduce(
                            out=M_hist[:, (s + 1) * K:(s + 2) * K],
                            in_=cand_f[:].rearrange("b (i j) -> b i j", i=K),
                            axis=AxX, op=Alu.max)

                # backward sweep (descending)
                pend = build_b(*chunks[-1])
                for ci in range(len(chunks) - 1, -1, -1):
                    s0, ns = chunks[ci]
                    sb_ = pend
                    if ci > 0:
                        pend = build_b(*chunks[ci - 1])
                    for k in range(ns - 1, -1, -1):
                        s = s0 + k   # beta_s from beta_{s+1} and B_b[s]
                        nc.vector.tensor_tensor(
                            out=cand_b[:].rearrange("b (i j) -> b i j", i=K),
                            in0=B_hist[:, (s + 1) * K:(s + 2) * K].unsqueeze(1)
                                .to_broadcast([Bc, K, K]),
                            in1=sview(sb_, ns)[:, k], op=Alu.add)
                        nc.vector.tensor_reduce(
                            out=B_hist[:, s * K:(s + 1) * K],
                            in_=cand_b[:].rearrange("b (i j) -> b i j", i=K),
                            axis=AxX, op=Alu.max)

            # ---------- P6: tags = argmax_i(M + em + beta) ----------
            tags_f = dbig.tile([Bc, T], F32, tag="tags_f")
            # sc aliases em2's allocation (em2 is dead after the bwd slab
            # builds); eq aliases the bigA slot (h_f is dead after P4).
            sc = dbig.tile([Bc, T * K], F32, tag="em2")
            nc.vector.tensor_tensor(
                out=sc[:], in0=M_hist[:], in1=B_hist[:], op=Alu.add)
            emv = em1[:].rearrange("b (k t) -> b t k", k=K)
            nc.vector.tensor_tensor(
                out=sc[:].rearrange("b (t k) -> b t k", k=K),
                in0=sc[:].rearrange("b (t k) -> b t k", k=K),
                in1=emv, op=Alu.add)
            mx = dbig.tile([Bc, T], F32, tag="mx")
            nc.vector.tensor_reduce(
                out=mx[:], in_=sc[:].rearrange("b (t k) -> b t k", k=K),
                axis=AxX, op=Alu.max)
            eq = big.tile([Bc, T * K], F32, tag="bigA")
            nc.vector.tensor_tensor(
                out=eq[:].rearrange("b (t k) -> b t k", k=K),
                in0=sc[:].rearrange("b (t k) -> b t k", k=K),
                in1=mx[:].unsqueeze(2).to_broadcast([Bc, T, K]),
                op=Alu.is_equal)
            irev_v = irev9_sb[:].unsqueeze(1).to_broadcast([Bc, T, K])
            nc.vector.tensor_tensor(
                out=eq[:].rearrange("b (t k) -> b t k", k=K),
                in0=eq[:].rearrange("b (t k) -> b t k", k=K),
                in1=irev_v, op=Alu.mult)
            nc.vector.tensor_reduce(
                out=tags_f[:], in_=eq[:].rearrange("b (t k) -> b t k", k=K),
                axis=AxX, op=Alu.max)
            nc.vector.tensor_scalar(
                out=tags_f[:], in0=tags_f[:], scalar1=-1.0, scalar2=8.0,
                op0=Alu.mult, op1=Alu.add)
            nc.vector.tensor_tensor(
                out=tags_f[:], in0=tags_f[:], in1=mask_sb[:], op=Alu.mult)
            tags_i = dbig.tile([Bc, T], I32, tag="tags_i")
            nc.vector.tensor_copy(out=tags_i[:], in_=tags_f[:])
            nc.sync.dma_start(out=tags_out[:], in_=tags_i[:])
            dpbig_cm.__exit__(None, None, None)
            psp_cm.__exit__(None, None, None)

    nc.finalize()
    return nc


_NC_CACHE = None


def _get_program():
    global _NC_CACHE
    if _NC_CACHE is None:
        _NC_CACHE = build_program()
    return _NC_CACHE


def make_in_maps(sentences, lengths, embed, Wih_f, Whh_f, bih_f, bhh_f,
                 Wih_b, Whh_b, bih_b, bhh_b, W_out, b_out, start_t, end_t,
                 trans):
    sentences = np.ascontiguousarray(sentences, dtype=np.int32)
    embed = np.ascontiguousarray(embed, dtype=np.float32)
    lengths = np.asarray(lengths)

    whh_pack = np.zeros((128, 1024), np.float32)
    wih_pack = np.zeros((128, 1024), np.float32)
    bias_pack = np.zeros((128, 8), np.float32)
    for d, (Wih, Whh, bi, bh) in enumerate(
            ((Wih_f, Whh_f, bih_f, bhh_f), (Wih_b, Whh_b, bih_b, bhh_b))):
        for g in range(G4):
            whh_pack[:, (d * G4 + g) * 128:(d * G4 + g + 1) * 128] = \
                np.asarray(Whh)[g * 128:(g + 1) * 128, :].T
            wih_pack[:, (d * G4 + g) * 128:(d * G4 + g + 1) * 128] = \
                np.asarray(Wih)[g * 128:(g + 1) * 128, :].T
            bias_pack[:, d * G4 + g] = \
                (np.asarray(bi) + np.asarray(bh))[g * 128:(g + 1) * 128]

    W_out = np.asarray(W_out, np.float32)
    woutT = np.zeros((128, 18), np.float32)
    woutT[:, 0:K] = W_out[:, :128].T
    woutT[:, K:2 * K] = W_out[:, 128:].T
    bout9 = np.asarray(b_out, np.float32)[:, None].copy()

    rev_id = np.zeros((128, 128), np.float32)
    rev_id[np.arange(128), 127 - np.arange(128)] = 1.0

    trans_np = np.asarray(trans, np.float32)
    identM = np.full((K, K), NEG, np.float32)
    np.fill_diagonal(identM, 0.0)
    transT9 = np.broadcast_to(trans_np.T.reshape(-1)[None], (Bc, 81)).copy()
    trans9 = np.broadcast_to(trans_np.reshape(-1)[None], (Bc, 81)).copy()
    identM9 = np.broadcast_to(identM.reshape(-1)[None], (Bc, 81)).copy()

    start9 = np.broadcast_to(
        np.asarray(start_t, np.float32)[None, :], (Bc, K)).copy()
    end9 = np.broadcast_to(
        np.asarray(end_t, np.float32)[None, :], (Bc, K)).copy()
    ii = np.arange(K, dtype=np.float32)
    irev9 = np.broadcast_to((8.0 - ii)[None, :], (Bc, K)).copy()
    tt = np.arange(T)

    in_maps = []
    for c in range(NC):
        sl = slice(c * Bc, (c + 1) * Bc)
        sents_c = sentences[sl]
        lens_c = np.asarray(lengths[sl], np.float32)
        idx_np = np.zeros((128, NBLK), np.int32)
        p = np.arange(128)
        for g in range(NBLK):
            bt = g * 128 + p
            idx_np[:, g] = sents_c[bt // T, bt % T]
        lens_bc = np.broadcast_to(lens_c[None, :], (128, Bc)).copy()
        mask_np = (tt[None, :] < lens_c[:, None]).astype(np.float32)
        whh_send = whh_pack.astype(ml_dtypes.bfloat16) if REC_BF16 else whh_pack
        in_maps.append({
            "embed": embed,
            "idx": idx_np,
            "whh_pack": whh_send, "wih_pack": wih_pack, "bias_pack": bias_pack,
            "lens_b": lens_bc,
            "woutT": woutT, "bout9": bout9, "rev_id": rev_id,
            "transT9": transT9, "trans9": trans9, "identM9": identM9,
            "start9": start9, "end9": end9, "irev9": irev9,
            "mask_dp": mask_np, "invm_dp": 1.0 - mask_np,
        })
    return in_maps


def run(inputs, trace=False, **kw):
    nc = _get_program()
    in_maps = make_in_maps(**inputs)
    res = run_bass_kernel_spmd(nc, in_maps, list(range(NC)), trace=trace, **kw)
    tags = np.concatenate([r["tags"] for r in res.results], axis=0)
    return tags.astype(np.int32), res


def kernel(**inputs):
    tags, _ = run(inputs)
    return tags
